# revision 21
# baseline (speedup 1.0000x reference)
"""Bass/Trainium2 kernel for nn_BayesianResNet_71408126263673.

Grouped per-sample conv: for each of 32 samples i,
  out[i] = conv2d(x[i] [128,32,32], W[i] [128oc,128c,3,3], pad=1, stride=1) + bias[i]

Sharding: b_i (32 samples) split across 8 NeuronCores, 4 samples per core.
Pure data parallel, no collectives.

Per-core kernel: each sample's conv is computed as 9 accumulating matmuls
(one per 3x3 tap) into PSUM:
  out[oc, pix] = sum_{kh,kw} W[:, :, kh, kw].T @ xpad[:, shifted pix]
with K=c=128 (partition/contraction), M=oc=128, N=512 pixels (16 output rows
per PSUM bank). The input image is zero-padded to 34x34 on the HOST so DMA
loads are fully contiguous. Weights are pre-transposed on the host to
[c, kh*kw, oc] so each tap is a ready-to-use lhsT (stationary) tile.

Schedule (measured ~32.0-33.0us vs 33.3us for the v1 single-queue layout):
- Weights ride the SP (sync) HWDGE queue while images ride the ACT (scalar)
  queue in parallel. Both queues share a slow ~4us aggregate ramp
  (~20 -> 350 KB/us), so sample 0's ~580KB gates the first real matmul at
  ~11us into the exec window regardless of arrangement; parallel queues +
  sample-0's image split top/bottom (so block-0 only waits for rows 0-17)
  sit at that wall. Later samples use one DMA job each (each extra job
  costs ~0.5us of queue overhead).
- ~30 junk warmup matmuls keep the PE busy from engine start so the HAM
  clock-gate reaches 2.4 GHz exactly when sample 0's data lands.
- Outputs are written as fp16 (halves store bytes; adds ~3e-4 rel err
  against the 2e-2 budget) and upcast on the host. The last sample's
  second half runs as two 8-row blocks with per-block stores on alternate
  queues, shortening the post-matmul tail by ~1us.
- The remaining structure is walled in: ~1.3us framework preamble, ~8.3us
  walrus postamble (per-sem resets of all 254 sems, Tensor-engine chain is
  the critical path), and a 72-matmul fp16 stream already at the 216ns
  N=512 issue floor. fp8 fails accuracy (3.9e-2); uint8+DoublePixel (the
  only 2x mode) is rejected by the BIR verifier; DoublePixel on fp8 is a
  throughput no-op (measured).
"""

import os
import numpy as np

import concourse.bacc as bacc
import concourse.tile as tile
from concourse import mybir
from concourse.bass_utils import run_bass_kernel_spmd

N_CORES = 8
B_I, B_J, C, H, W = 32, 1, 128, 32, 32
OC, KH, KW = 128, 3, 3
S = B_I // N_CORES            # samples per core
HP, WP = H + 2, W + 2         # padded image
NTAP = KH * KW                # 9
NBLK = 2                      # output row blocks per sample
RPB = H // NBLK               # 16 rows per block -> N = 512
XTOP = 18 * WP                # image cols covering output rows 0-15 (input rows 0-17)

_DT_TABLE = {
    "fp32": (mybir.dt.float32, np.float32),
    "fp32r": (mybir.dt.float32r, np.float32),
    "fp16": (mybir.dt.float16, np.float16),
}
_MM_DT_NAME = os.environ.get("CONV_MM_DTYPE", "fp16")
MM_DT, MM_NP = _DT_TABLE[_MM_DT_NAME]

F16 = mybir.dt.float16
F32 = mybir.dt.float32
X_DT = W_DT = MM_DT  # test.py print compatibility

# test.py hooks: set TRACE=True before calling kernel() to profile; the
# BassKernelResults of the last run lands in LAST_RESULTS.
TRACE = False
TRACE_KW = {}
LAST_RESULTS = None

_NC_CACHE = None


def _build_nc():
    nc = bacc.Bacc()
    w_d = nc.declare_dram_parameter("w", [S, C, NTAP * OC], MM_DT, isOutput=False)
    x_d = nc.declare_dram_parameter("x", [S, C, HP * WP], MM_DT, isOutput=False)
    b_d = nc.declare_dram_parameter("b", [OC, S], F32, isOutput=False)
    o_d = nc.declare_dram_parameter("o", [S, OC, H, W], F16, isOutput=True)

    with tile.TileContext(nc, pool_alloc_mode="queue") as tc:
        with (
            tc.tile_pool(name="ins", bufs=1) as ins_pool,
            tc.tile_pool(name="outs", bufs=1) as outs_pool,
            tc.tile_pool(name="psum", bufs=8, space="PSUM") as psum_pool,
        ):
            # PE warmup: dependency-free matmuls on a vector-memset tile keep
            # the PE busy from engine start so the HAM clock-gate reaches
            # 2.4 GHz before the first real matmul. DVE finishes its preamble
            # ~1.3us before GpSimd, so memset there to start warmup ASAP.
            wu_x = ins_pool.tile([C, 2 * OC], MM_DT, tag="warmup", name="warmup")
            nc.vector.memset(wu_x[:], 0.0)
            wu_ps = psum_pool.tile([C, OC], F32, name="wu_ps", tag="ps")
            for _ in range(30):
                nc.tensor.matmul(
                    wu_ps[:], wu_x[:, :OC], wu_x[:, OC:], start=True, stop=True
                )

            w_ts = [ins_pool.tile([C, NTAP * OC], MM_DT, tag=f"w{s}", name=f"w{s}") for s in range(S)]
            x_ts = [ins_pool.tile([C, HP * WP], MM_DT, tag=f"x{s}", name=f"x{s}") for s in range(S)]
            xvs = [t[:].rearrange("p (h w) -> p h w", w=WP) for t in x_ts]
            bias_t = ins_pool.tile([OC, S], F32, tag="bias")

            # Weights on the SP queue, images on the ACT queue, in sample
            # order so both halves of sample s land at about the same time.
            # The two queues share a slow ~3us aggregate ramp (~20 -> 350
            # KB/us), so sample 0's ~580KB cannot land before ~11us however
            # arranged; parallel queues with tap-0 weights first is optimal.
            # Sample 0's weights go as two jobs: a DMA job's completion
            # semaphore only fires once the WHOLE job lands (+~0.7-1.2us
            # propagation), and the first LDWEIGHTS waits on it. Splitting
            # taps 0-4 into their own job unblocks the first matmuls ~0.3us
            # earlier (A/B-verified) and decouples the start from tail
            # hiccups in the big transfer; taps 5-8 arrive before they are
            # consumed at the 216ns/matmul pace.
            nc.sync.dma_start(w_ts[0][:, : 5 * OC], w_d[0][:, : 5 * OC])
            # First 8-row block of sample 0 only needs x rows 0-9; give
            # them their own job so its completion sem fires earliest
            # (A/B-verified ~0.55us win over one rows-0-17 job).
            XA = 10 * WP
            nc.scalar.dma_start(x_ts[0][:, :XA], x_d[0][:, :XA])
            nc.scalar.dma_start(x_ts[0][:, XA:XTOP], x_d[0][:, XA:XTOP])
            nc.sync.dma_start(w_ts[0][:, 5 * OC :], w_d[0][:, 5 * OC :])
            nc.scalar.dma_start(x_ts[0][:, XTOP:], x_d[0][:, XTOP:])
            nc.sync.dma_start(bias_t[:], b_d[:])
            for s in range(1, S):
                # Single x job per later sample: each extra DMA job costs
                # ~0.5us of queue overhead and these aren't start-critical.
                nc.sync.dma_start(w_ts[s][:], w_d[s])
                nc.scalar.dma_start(x_ts[s][:], x_d[s])

            def conv_block(s, row0, nrows, ps_name):
                """One accumulation group: output rows [row0, row0+nrows)."""
                ps = psum_pool.tile([OC, nrows, W], F32, name=ps_name, tag="ps")
                for t in range(NTAP):
                    kh, kw = divmod(t, KW)
                    rhs = xvs[s][:, row0 + kh : row0 + kh + nrows, kw : kw + W]
                    lhsT = w_ts[s][:, t * OC : (t + 1) * OC]
                    nc.tensor.matmul(
                        ps[:], lhsT, rhs, start=(t == 0), stop=(t == NTAP - 1)
                    )
                return ps

            for s in range(S):
                out_t = outs_pool.tile([OC, H, W], F16, tag=f"out{s}", name=f"out{s}")
                if s == 0:
                    # Two 8-row blocks first (same PE cost as one 16-row
                    # block) so the first matmuls wait only for x rows 0-9.
                    blocks = ((0, RPB // 2), (RPB // 2, RPB // 2), (RPB, RPB))
                elif s < S - 1:
                    blocks = ((0, RPB), (RPB, RPB))
                else:
                    # Last sample: shrink the final act+store so the kernel's
                    # tail (after the last matmul) is as short as possible.
                    blocks = ((0, RPB), (RPB, RPB // 2), (RPB + RPB // 2, RPB // 2))
                for bi, (row0, nrows) in enumerate(blocks):
                    ps = conv_block(s, row0, nrows, f"ps{s}_{bi}")
                    nc.scalar.activation(
                        out_t[:, row0 : row0 + nrows, :],
                        ps[:],
                        mybir.ActivationFunctionType.Identity,
                        bias=bias_t[:, s : s + 1],
                    )
                    if s == S - 1:
                        # Per-block stores on alternating queues so the last
                        # pieces stream in parallel.
                        eng = nc.scalar if bi % 2 == 0 else nc.sync
                        eng.dma_start(
                            o_d[s][:, row0 : row0 + nrows, :],
                            out_t[:, row0 : row0 + nrows, :],
                        )
                if s < S - 1:
                    # Alternate store queues so neither backs up.
                    eng = nc.scalar if s % 2 == 0 else nc.sync
                    eng.dma_start(o_d[s], out_t[:])
    nc.compile()
    return nc


def _get_nc():
    global _NC_CACHE
    if _NC_CACHE is None:
        _NC_CACHE = _build_nc()
    return _NC_CACHE


def kernel(x: np.ndarray, weight: np.ndarray, bias: np.ndarray) -> np.ndarray:
    global LAST_RESULTS
    assert x.shape == (B_I, B_J, C, H, W)
    assert weight.shape == (B_I, OC, C, KH, KW)
    assert bias.shape == (B_I, B_J, OC)

    x = np.asarray(x, dtype=np.float32)
    weight = np.asarray(weight, dtype=np.float32)
    bias = np.asarray(bias, dtype=np.float32)

    # Host-side layout prep (part of sharding): zero-pad images, transpose
    # weights so each 3x3 tap is a contiguous [c, oc] stationary tile.
    wt = np.ascontiguousarray(weight.transpose(0, 2, 3, 4, 1))  # [b_i, c, kh, kw, oc]
    wt = wt.reshape(B_I, C, NTAP * OC).astype(MM_NP)
    xpad = np.zeros((B_I, C, HP, WP), dtype=MM_NP)
    xpad[:, :, 1 : 1 + H, 1 : 1 + W] = x[:, 0].astype(MM_NP)
    xpad = xpad.reshape(B_I, C, HP * WP)
    bt = bias[:, 0, :]  # [b_i, oc]

    in_maps = []
    for core in range(N_CORES):
        sl = slice(core * S, (core + 1) * S)
        in_maps.append(
            {
                "w": np.ascontiguousarray(wt[sl]),
                "x": np.ascontiguousarray(xpad[sl]),
                "b": np.ascontiguousarray(bt[sl].T),  # [OC, S]
            }
        )

    nc = _get_nc()
    try:
        res = run_bass_kernel_spmd(
            nc, in_maps, core_ids=list(range(N_CORES)), trace=TRACE, **TRACE_KW
        )
    except Exception:
        # Transient NRT/device errors (e.g. NRT_EXEC_UNIT_UNRECOVERABLE after
        # heavy reuse) usually clear on retry; the work is idempotent.
        import time

        time.sleep(10)
        res = run_bass_kernel_spmd(
            nc, in_maps, core_ids=list(range(N_CORES)), trace=TRACE, **TRACE_KW
        )
    LAST_RESULTS = res

    out = np.concatenate(
        [np.asarray(res.results[c]["o"]).astype(np.float32) for c in range(N_CORES)],
        axis=0,
    )
    return out.reshape(B_I, B_J, OC, H, W)


# revision 22
# speedup vs baseline: 1.0374x; 1.0374x over previous
"""Bass/Trainium2 kernel for nn_BayesianResNet_71408126263673.

Grouped per-sample conv: for each of 32 samples i,
  out[i] = conv2d(x[i] [128,32,32], W[i] [128oc,128c,3,3], pad=1, stride=1) + bias[i]

Sharding: b_i (32 samples) split across 8 NeuronCores, 4 samples per core.
Pure data parallel, no collectives.

Per-core kernel: each sample's conv is computed as 9 accumulating matmuls
(one per 3x3 tap) into PSUM:
  out[oc, pix] = sum_{kh,kw} W[:, :, kh, kw].T @ xpad[:, shifted pix]
with K=c=128 (partition/contraction), M=oc=128, N=512 pixels (16 output rows
per PSUM bank). The input image is zero-padded to 34x34 on the HOST so DMA
loads are fully contiguous. Weights are pre-transposed on the host to
[c, kh*kw, oc] so each tap is a ready-to-use lhsT (stationary) tile.

Schedule (measured ~32.0-33.0us vs 33.3us for the v1 single-queue layout):
- Weights ride the SP (sync) HWDGE queue while images ride the ACT (scalar)
  queue in parallel. Both queues share a slow ~4us aggregate ramp
  (~20 -> 350 KB/us), so sample 0's ~580KB gates the first real matmul at
  ~11us into the exec window regardless of arrangement; parallel queues +
  sample-0's image split top/bottom (so block-0 only waits for rows 0-17)
  sit at that wall. Later samples use one DMA job each (each extra job
  costs ~0.5us of queue overhead).
- ~30 junk warmup matmuls keep the PE busy from engine start so the HAM
  clock-gate reaches 2.4 GHz exactly when sample 0's data lands.
- Outputs are written as fp16 (halves store bytes; adds ~3e-4 rel err
  against the 2e-2 budget) and upcast on the host. The last sample's
  second half runs as two 8-row blocks with per-block stores on alternate
  queues, shortening the post-matmul tail by ~1us.
- The remaining structure is walled in: ~1.3us framework preamble, ~8.3us
  walrus postamble (per-sem resets of all 254 sems, Tensor-engine chain is
  the critical path), and a 72-matmul fp16 stream already at the 216ns
  N=512 issue floor. fp8 fails accuracy (3.9e-2); uint8+DoublePixel (the
  only 2x mode) is rejected by the BIR verifier; DoublePixel on fp8 is a
  throughput no-op (measured).
"""

import os
import numpy as np

import concourse.bacc as bacc
import concourse.tile as tile
from concourse import mybir
from concourse.bass_utils import run_bass_kernel_spmd

N_CORES = 8
B_I, B_J, C, H, W = 32, 1, 128, 32, 32
OC, KH, KW = 128, 3, 3
S = B_I // N_CORES            # samples per core
HP, WP = H + 2, W + 2         # padded image
NTAP = KH * KW                # 9
NBLK = 2                      # output row blocks per sample
RPB = H // NBLK               # 16 rows per block -> N = 512
XTOP = 18 * WP                # image cols covering output rows 0-15 (input rows 0-17)

_DT_TABLE = {
    "fp32": (mybir.dt.float32, np.float32),
    "fp32r": (mybir.dt.float32r, np.float32),
    "fp16": (mybir.dt.float16, np.float16),
}
_MM_DT_NAME = os.environ.get("CONV_MM_DTYPE", "fp16")
MM_DT, MM_NP = _DT_TABLE[_MM_DT_NAME]

F16 = mybir.dt.float16
F32 = mybir.dt.float32
X_DT = W_DT = MM_DT  # test.py print compatibility

# test.py hooks: set TRACE=True before calling kernel() to profile; the
# BassKernelResults of the last run lands in LAST_RESULTS.
TRACE = False
TRACE_KW = {}
LAST_RESULTS = None

_NC_CACHE = None


def _build_nc():
    nc = bacc.Bacc()
    w_d = nc.declare_dram_parameter("w", [S, C, NTAP * OC], MM_DT, isOutput=False)
    x_d = nc.declare_dram_parameter("x", [S, C, HP * WP], MM_DT, isOutput=False)
    b_d = nc.declare_dram_parameter("b", [OC, S], F32, isOutput=False)
    o_d = nc.declare_dram_parameter("o", [S, OC, H, W], F16, isOutput=True)

    with tile.TileContext(nc, pool_alloc_mode="queue") as tc:
        with (
            tc.tile_pool(name="ins", bufs=1) as ins_pool,
            tc.tile_pool(name="outs", bufs=1) as outs_pool,
            tc.tile_pool(name="psum", bufs=8, space="PSUM") as psum_pool,
        ):
            # PE warmup: dependency-free matmuls on a vector-memset tile keep
            # the PE busy from engine start so the HAM clock-gate reaches
            # 2.4 GHz before the first real matmul. DVE finishes its preamble
            # ~1.3us before GpSimd, so memset there to start warmup ASAP.
            wu_x = ins_pool.tile([C, 2 * OC], MM_DT, tag="warmup", name="warmup")
            nc.vector.memset(wu_x[:], 0.0)
            wu_ps = psum_pool.tile([C, OC], F32, name="wu_ps", tag="ps")
            for _ in range(30):
                nc.tensor.matmul(
                    wu_ps[:], wu_x[:, :OC], wu_x[:, OC:], start=True, stop=True
                )

            w_ts = [ins_pool.tile([C, NTAP * OC], MM_DT, tag=f"w{s}", name=f"w{s}") for s in range(S)]
            x_ts = [ins_pool.tile([C, HP * WP], MM_DT, tag=f"x{s}", name=f"x{s}") for s in range(S)]
            xvs = [t[:].rearrange("p (h w) -> p h w", w=WP) for t in x_ts]
            bias_t = ins_pool.tile([OC, S], F32, tag="bias")

            # Weights on the SP queue, images on the ACT queue, in sample
            # order so both halves of sample s land at about the same time.
            # The two queues share a slow ~3us aggregate ramp (~20 -> 350
            # KB/us), so sample 0's ~580KB cannot land before ~11us however
            # arranged; parallel queues with tap-0 weights first is optimal.
            # Sample 0's weights go as two jobs: a DMA job's completion
            # semaphore only fires once the WHOLE job lands (+~0.7-1.2us
            # propagation), and the first LDWEIGHTS waits on it. Splitting
            # taps 0-4 into their own job unblocks the first matmuls ~0.3us
            # earlier (A/B-verified) and decouples the start from tail
            # hiccups in the big transfer; taps 5-8 arrive before they are
            # consumed at the 216ns/matmul pace.
            nc.sync.dma_start(w_ts[0][:, : 5 * OC], w_d[0][:, : 5 * OC])
            # First 8-row block of sample 0 only needs x rows 0-9; give
            # them their own job so its completion sem fires earliest
            # (A/B-verified ~0.55us win over one rows-0-17 job).
            XA = 10 * WP
            nc.scalar.dma_start(x_ts[0][:, :XA], x_d[0][:, :XA])
            nc.scalar.dma_start(x_ts[0][:, XA:XTOP], x_d[0][:, XA:XTOP])
            nc.sync.dma_start(w_ts[0][:, 5 * OC :], w_d[0][:, 5 * OC :])
            nc.scalar.dma_start(x_ts[0][:, XTOP:], x_d[0][:, XTOP:])
            nc.sync.dma_start(bias_t[:], b_d[:])
            for s in range(1, S):
                # Single x job per later sample: each extra DMA job costs
                # ~0.5us of queue overhead and these aren't start-critical.
                nc.sync.dma_start(w_ts[s][:], w_d[s])
                nc.scalar.dma_start(x_ts[s][:], x_d[s])

            def conv_block(s, row0, nrows, ps_name):
                """One accumulation group: output rows [row0, row0+nrows)."""
                ps = psum_pool.tile([OC, nrows, W], F32, name=ps_name, tag="ps")
                for t in range(NTAP):
                    kh, kw = divmod(t, KW)
                    rhs = xvs[s][:, row0 + kh : row0 + kh + nrows, kw : kw + W]
                    lhsT = w_ts[s][:, t * OC : (t + 1) * OC]
                    nc.tensor.matmul(
                        ps[:], lhsT, rhs, start=(t == 0), stop=(t == NTAP - 1)
                    )
                return ps

            for s in range(S):
                out_t = outs_pool.tile([OC, H, W], F16, tag=f"out{s}", name=f"out{s}")
                if s == 0:
                    # Two 8-row blocks first (same PE cost as one 16-row
                    # block) so the first matmuls wait only for x rows 0-9.
                    blocks = ((0, RPB // 2), (RPB // 2, RPB // 2), (RPB, RPB))
                elif s < S - 1:
                    blocks = ((0, RPB), (RPB, RPB))
                else:
                    # Last sample: shrink the final act+store so the kernel's
                    # tail (after the last matmul) is as short as possible.
                    blocks = ((0, RPB), (RPB, RPB // 2), (RPB + RPB // 2, RPB // 2))
                for bi, (row0, nrows) in enumerate(blocks):
                    ps = conv_block(s, row0, nrows, f"ps{s}_{bi}")
                    nc.scalar.activation(
                        out_t[:, row0 : row0 + nrows, :],
                        ps[:],
                        mybir.ActivationFunctionType.Identity,
                        bias=bias_t[:, s : s + 1],
                    )
                    if s == S - 1:
                        # Keep the scalar engine free to run the three acts
                        # back-to-back (store dispatches cost ~600ns and
                        # would wedge between them); only the final store
                        # goes on scalar, right behind its act with no
                        # cross-engine sem hop.
                        eng = nc.scalar if bi == len(blocks) - 1 else nc.sync
                        eng.dma_start(
                            o_d[s][:, row0 : row0 + nrows, :],
                            out_t[:, row0 : row0 + nrows, :],
                        )
                if s < S - 1:
                    # Alternate store queues so neither backs up.
                    eng = nc.scalar if s % 2 == 0 else nc.sync
                    eng.dma_start(o_d[s], out_t[:])
    nc.compile()
    return nc


def _get_nc():
    global _NC_CACHE
    if _NC_CACHE is None:
        _NC_CACHE = _build_nc()
    return _NC_CACHE


def kernel(x: np.ndarray, weight: np.ndarray, bias: np.ndarray) -> np.ndarray:
    global LAST_RESULTS
    assert x.shape == (B_I, B_J, C, H, W)
    assert weight.shape == (B_I, OC, C, KH, KW)
    assert bias.shape == (B_I, B_J, OC)

    x = np.asarray(x, dtype=np.float32)
    weight = np.asarray(weight, dtype=np.float32)
    bias = np.asarray(bias, dtype=np.float32)

    # Host-side layout prep (part of sharding): zero-pad images, transpose
    # weights so each 3x3 tap is a contiguous [c, oc] stationary tile.
    wt = np.ascontiguousarray(weight.transpose(0, 2, 3, 4, 1))  # [b_i, c, kh, kw, oc]
    wt = wt.reshape(B_I, C, NTAP * OC).astype(MM_NP)
    xpad = np.zeros((B_I, C, HP, WP), dtype=MM_NP)
    xpad[:, :, 1 : 1 + H, 1 : 1 + W] = x[:, 0].astype(MM_NP)
    xpad = xpad.reshape(B_I, C, HP * WP)
    bt = bias[:, 0, :]  # [b_i, oc]

    in_maps = []
    for core in range(N_CORES):
        sl = slice(core * S, (core + 1) * S)
        in_maps.append(
            {
                "w": np.ascontiguousarray(wt[sl]),
                "x": np.ascontiguousarray(xpad[sl]),
                "b": np.ascontiguousarray(bt[sl].T),  # [OC, S]
            }
        )

    nc = _get_nc()
    try:
        res = run_bass_kernel_spmd(
            nc, in_maps, core_ids=list(range(N_CORES)), trace=TRACE, **TRACE_KW
        )
    except Exception:
        # Transient NRT/device errors (e.g. NRT_EXEC_UNIT_UNRECOVERABLE after
        # heavy reuse) usually clear on retry; the work is idempotent.
        import time

        time.sleep(10)
        res = run_bass_kernel_spmd(
            nc, in_maps, core_ids=list(range(N_CORES)), trace=TRACE, **TRACE_KW
        )
    LAST_RESULTS = res

    out = np.concatenate(
        [np.asarray(res.results[c]["o"]).astype(np.float32) for c in range(N_CORES)],
        axis=0,
    )
    return out.reshape(B_I, B_J, OC, H, W)
